# revision 14
# baseline (speedup 1.0000x reference)
"""Chamfer distance kernel for 8 Trainium2 NeuronCores (Bass/Tile).

Problem: pc1, pc2: [2, 8192, 3] f32.
  dist[b,n,m] = ||pc1[b,n]-pc2[b,m]||^2
  out = mean_n(min_m dist) + mean_m(min_n dist)   (scalar f32)

Single-pass strategy (v2):
  * Augmented-matmul: dist[n,m] = L1[:,n] . R2[:,m] with K=20 bf16 hi/lo
    split (fp32-accurate distances straight into PSUM).
  * Each core owns 1/8 of pc1's rows and computes its [1024, 8192] block
    of the distance matrix ONCE (half the PE work of the two-pass scheme).
    - dist1 rows for the shard are complete: row-min over the free axis.
    - dist2 needs column mins: partial per core, combined on host.
  * All reduction work happens in NEGATED space (-d) because the gpsimd
    partition_all_reduce endgame only supports max:
    - ACT evacuates each PSUM tile with scale=-1 to bf16 SBUF (sc = -d).
    - DVE: row-max hierarchy per row-block (pairwise bf16 folds at the 2x
      rate + one narrow reduce)  ->  -dist1 columns.
    - DVE/Pool split: col-max accumulators cacc[b,g] = max over row blocks
      (per-tile tensor_tensor max; bf16 2x on DVE, Pool takes some groups
      to balance engines).
    - Pool: partition_all_reduce(max) per cacc -> negated partial col-mins,
      one row DMA'd out per (batch, group).
  * Host: negate, min over cores for dist2, then means in fp64.

Engine budget per core (cost-model): ACT ~121us, DVE ~113us, Pool ~106us,
PE ~55us  (baseline two-pass was DVE-bound at ~287us).
"""

from contextlib import ExitStack

import numpy as np

import concourse.bass as bass
import concourse.tile as tile
from concourse import bacc, bass_isa, mybir
from concourse.bass_utils import run_bass_kernel_spmd

B = 2
N = 8192  # pc1 points per batch
M = 8192  # pc2 points per batch
NCORES = 8
NLOC = N // NCORES  # 1024 pc1 rows per core
NRB = NLOC // 128  # 8 row blocks per core
GW = 2048  # psum tile free width (4 banks); 2 bufs = all 8 banks
NG = M // GW  # 4 column groups

# kept for test.py compatibility (modes are baked into this kernel now)
MATMUL_MODE = "bf16"
REDUCE_MODE = "spass"
MM_W = 512

K = 20  # 5 augmented features x4 (bf16 hi/lo on both operands)
BF16 = mybir.dt.bfloat16
F32 = mybir.dt.float32
NEG_INF = -3.0e38

# column groups whose col-max fold chain runs on the Pool engine
POOL_GS = (2, 3)


def _build_nc(mode=MATMUL_MODE, reps=1, reduce_mode=REDUCE_MODE, mm_w=MM_W,
              pool_gs=POOL_GS):
    nc = bacc.Bacc("TRN2", target_bir_lowering=False, debug=False,
                   num_devices=NCORES)

    al = nc.dram_tensor("al", [B, K, NLOC], BF16, kind="ExternalInput")
    br = nc.dram_tensor("br", [B, K, M], BF16, kind="ExternalInput")
    # d1c[p, b*NRB+rb] = -min_m dist[b, rb*128+p, m]
    d1c = nc.dram_tensor("d1c", [128, B * NRB], BF16, kind="ExternalOutput")
    # d2c rows: negated partial col-mins, one per (batch, group, rb-chunk).
    # b=0 has 4 chunks (rb pairs) per group; b=1 has 5 (last two rbs are
    # singletons so the final par-reduces don't wait on folds).
    # row = b*16 + g*npair_b + chunk. Host maxes over chunks and cores.
    d2c = nc.dram_tensor("d2c", [NG * 9, GW], BF16, kind="ExternalOutput")

    MAX = mybir.AluOpType.max

    with tile.TileContext(nc) as tc, ExitStack() as ctx:
        sb = ctx.enter_context(tc.tile_pool(name="sb", bufs=1))
        ps = ctx.enter_context(tc.tile_pool(name="ps", bufs=2, space="PSUM"))
        scp = ctx.enter_context(tc.tile_pool(name="scp", bufs=2))
        hierp = ctx.enter_context(tc.tile_pool(name="hierp", bufs=2))
        outp = ctx.enter_context(tc.tile_pool(name="outp", bufs=2))

        def body():
            # ---- inputs -> SBUF ----
            al_sb, br_sb = {}, {}
            for b in range(B):
                t = sb.tile([K, NLOC], BF16, name=f"al{b}", tag=f"al{b}", bufs=2)
                nc.sync.dma_start(t[:], al.ap()[b])
                al_sb[b] = t
                t = sb.tile([K, M], BF16, name=f"br{b}", tag=f"br{b}", bufs=2)
                for g in range(NG):
                    nc.sync.dma_start(t[:, g * GW:(g + 1) * GW],
                                      br.ap()[b, :, g * GW:(g + 1) * GW])
                br_sb[b] = t

            # pre-warm the ACT function table during the input DMA so the
            # first real evacuation doesn't pay the ~2.7us table load
            warm = sb.tile([128, 16], BF16, name="warm", tag="warm")
            nc.vector.memset(warm[:], 0.0)
            nc.scalar.mul(warm[:], warm[:], -1.0)

            d1cols = sb.tile([128, B * NRB], BF16, name="d1cols", tag="d1cols")

            # ---- main loop: one [128, 8192] row-block at a time ----
            # Row blocks are processed in pairs: the even rb's evacuation
            # installs a fresh col-max accumulator per group (no DVE work),
            # the odd rb folds into it on DVE, then Pool immediately
            # partition-reduces the pair and the row is DMA'd out.
            cacc = {}
            for b in range(B):
                for rb in range(NRB):
                    lhsT = al_sb[b][:, rb * 128:(rb + 1) * 128]
                    single = (b == 1 and rb >= 6)
                    pair = rb - 3 if single else rb // 2
                    install = (rb % 2 == 0) or single
                    finish = (rb % 2 == 1) or single
                    scs = []
                    for g in range(NG):
                        pt = ps.tile([128, GW], F32, name="pt", tag="pt")
                        for j in range(GW // mm_w):
                            off = g * GW + j * mm_w
                            nc.tensor.matmul(
                                pt[:, j * mm_w:(j + 1) * mm_w],
                                lhsT,
                                br_sb[b][:, off:off + mm_w],
                            )
                        if install:
                            # negated evacuation doubles as accumulator.
                            # rb==0 runs on DVE (otherwise idle at start),
                            # the rest on ACT.
                            sc = scp.tile([128, GW], BF16, name=f"cacc{g}",
                                          tag=f"cacc{g}", bufs=3)
                            if rb == 0 and b == 0 and g >= 2:
                                nc.vector.tensor_scalar_mul(sc[:], pt[:], -1.0)
                            else:
                                nc.scalar.mul(sc[:], pt[:], -1.0)
                            cacc[g] = sc
                        else:
                            sc = scp.tile([128, GW], BF16, name=f"sc{g}",
                                          tag=f"sc{g}", bufs=2)
                            nc.scalar.mul(sc[:], pt[:], -1.0)
                            # DVE: fold odd rb into the pair accumulator
                            nc.vector.tensor_tensor(cacc[g][:], sc[:],
                                                    cacc[g][:], op=MAX)
                        scs.append(sc)

                    # DVE: row-max hierarchy for this row block
                    q1 = hierp.tile([128, GW], BF16, name="q1", tag="q1")
                    q2 = hierp.tile([128, GW], BF16, name="q2", tag="q2")
                    nc.vector.tensor_tensor(q1[:], scs[0][:], scs[1][:], op=MAX)
                    nc.vector.tensor_tensor(q2[:], scs[2][:], scs[3][:], op=MAX)
                    nc.vector.tensor_tensor(q1[:], q1[:], q2[:], op=MAX)
                    h1 = hierp.tile([128, 1024], BF16, name="h1", tag="h1")
                    nc.vector.tensor_tensor(h1[:], q1[:, 0:1024],
                                            q1[:, 1024:2048], op=MAX)
                    h2 = hierp.tile([128, 512], BF16, name="h2", tag="h2")
                    nc.vector.tensor_tensor(h2[:], h1[:, 0:512],
                                            h1[:, 512:1024], op=MAX)
                    h3 = hierp.tile([128, 256], BF16, name="h3", tag="h3")
                    nc.vector.tensor_tensor(h3[:], h2[:, 0:256],
                                            h2[:, 256:512], op=MAX)
                    col = b * NRB + rb
                    nc.vector.tensor_reduce(
                        d1cols[:, col:col + 1], h3[:],
                        axis=mybir.AxisListType.X, op=MAX)

                    if finish:
                        # Pool endgame for the completed chunk, interleaved
                        npair_b = 5 if b == 1 else 4
                        for g in range(NG):
                            pm = outp.tile([128, GW], BF16, name="pm",
                                           tag="pm")
                            nc.gpsimd.partition_all_reduce(
                                pm[:], cacc[g][:], channels=128,
                                reduce_op=bass_isa.ReduceOp.max)
                            row = b * 16 + g * npair_b + pair
                            nc.sync.dma_start(d2c.ap()[row], pm[0:1, :])

            nc.sync.dma_start(d1c.ap(), d1cols[:])

        if reps == 1:
            body()
        else:
            with tc.For_i(0, reps, 1):
                body()

    nc.compile()
    return nc


_NC_CACHE = {}


def _get_nc(mode=MATMUL_MODE, reps=1, reduce_mode=REDUCE_MODE, mm_w=MM_W):
    key = (mode, reps, reduce_mode, mm_w)
    if key not in _NC_CACHE:
        _NC_CACHE[key] = _build_nc(mode, reps, reduce_mode, mm_w)
    return _NC_CACHE[key]


def _lform(p):  # [B, n, 3] -> [B, 5, n]
    sq = (p * p).sum(-1)
    one = np.ones_like(sq)
    return np.stack([-2 * p[..., 0], -2 * p[..., 1], -2 * p[..., 2], sq, one],
                    axis=1)


def _rform(p):
    sq = (p * p).sum(-1)
    one = np.ones_like(sq)
    return np.stack([p[..., 0], p[..., 1], p[..., 2], one, sq], axis=1)


def _split_bf16(x):
    import ml_dtypes

    hi = x.astype(ml_dtypes.bfloat16).astype(np.float32)
    lo = (x - hi).astype(ml_dtypes.bfloat16).astype(np.float32)
    return hi, lo


def _pack(x, role):
    """f32 [B,5,n] -> matmul operand [B,20,n] bf16 (hi/lo product split)."""
    import ml_dtypes

    hi, lo = _split_bf16(x)
    if role == "l":
        out = np.concatenate([hi, hi, lo, lo], axis=1)
    else:
        out = np.concatenate([hi, lo, hi, lo], axis=1)
    return np.ascontiguousarray(out.astype(ml_dtypes.bfloat16))


def _make_in_maps(pc1, pc2, mode=MATMUL_MODE):
    L1 = _lform(pc1)
    R2 = _rform(pc2)
    L1p = _pack(L1, "l")
    brp = _pack(R2, "r")
    in_maps = []
    for c in range(NCORES):
        in_maps.append({
            "al": np.ascontiguousarray(L1p[:, :, c * NLOC:(c + 1) * NLOC]),
            "br": brp,
        })
    return in_maps


def kernel(pc1, pc2):
    pc1 = np.asarray(pc1, dtype=np.float32)
    pc2 = np.asarray(pc2, dtype=np.float32)
    assert pc1.shape == (B, N, 3) and pc2.shape == (B, M, 3)

    in_maps = _make_in_maps(pc1, pc2)
    nc = _get_nc()
    res = run_bass_kernel_spmd(nc, in_maps, list(range(NCORES)))

    # d1: negate, reassemble [B, N]
    d1 = np.empty((B, N), dtype=np.float64)
    for c in range(NCORES):
        d1c = np.asarray(res.results[c]["d1c"], dtype=np.float64)  # [128, B*NRB]
        for b in range(B):
            for rb in range(NRB):
                d1[b, c * NLOC + rb * 128:(c * NLOC) + (rb + 1) * 128] = \
                    -d1c[:, b * NRB + rb]

    # d2: max over cores and row-block chunks of negated partials, then negate
    d2n = np.full((B, M), -np.inf, dtype=np.float64)
    for c in range(NCORES):
        d2c = np.asarray(res.results[c]["d2c"], dtype=np.float64)
        for b, npair_b in ((0, 4), (1, 5)):
            part = d2c[b * 16:b * 16 + NG * npair_b]
            part = part.reshape(NG, npair_b, GW).max(axis=1)  # [NG, GW]
            np.maximum(d2n[b], part.reshape(M), out=d2n[b])
    d2 = -d2n

    out = d1.mean() + d2.mean()
    return np.float32(out)


# revision 16
# speedup vs baseline: 1.7467x; 1.7467x over previous
"""Chamfer distance kernel for 8 Trainium2 NeuronCores (Bass/Tile).

Problem: pc1, pc2: [2, 8192, 3] f32.
  dist[b,n,m] = ||pc1[b,n]-pc2[b,m]||^2
  out = mean_n(min_m dist) + mean_m(min_n dist)   (scalar f32)

Single-pass strategy (v2):
  * Augmented-matmul: dist[n,m] = L1[:,n] . R2[:,m] with K=20 bf16 hi/lo
    split (fp32-accurate distances straight into PSUM).
  * Each core owns 1/8 of pc1's rows and computes its [1024, 8192] block
    of the distance matrix ONCE (half the PE work of the two-pass scheme).
    - dist1 rows for the shard are complete: row-min over the free axis.
    - dist2 needs column mins: partial per core, combined on host.
  * All reduction work happens in NEGATED space (-d) because the gpsimd
    partition_all_reduce endgame only supports max:
    - ACT evacuates each PSUM tile with scale=-1 to bf16 SBUF (sc = -d);
      for even ("install") row blocks the evacuation doubles as a fresh
      per-pair col-max accumulator, so no separate DVE copy is needed.
    - DVE: folds the odd row block into the pair accumulator (bf16
      tensor_tensor max at the 2x rate) and computes the row-max hierarchy
      per row block (pairwise folds + one narrow reduce) -> -dist1 columns.
    - Pool: partition_all_reduce(max) per pair accumulator, interleaved
      with the main loop -> negated partial col-mins, one row per
      (batch, group, pair) DMA'd out.
  * Host: negate, max-combine pairs/cores for dist2, then means in fp64.

Engine budget per core (cost-model): ACT ~119us, DVE ~118us, Pool ~95us,
PE ~55us; sim total 137us (baseline two-pass was DVE-bound at ~297us).
"""

from contextlib import ExitStack

import numpy as np

import concourse.bass as bass
import concourse.tile as tile
from concourse import bacc, bass_isa, mybir
from concourse.bass_utils import run_bass_kernel_spmd

B = 2
N = 8192  # pc1 points per batch
M = 8192  # pc2 points per batch
NCORES = 8
NLOC = N // NCORES  # 1024 pc1 rows per core
NRB = NLOC // 128  # 8 row blocks per core
GW = 2048  # psum tile free width (4 banks); 2 bufs = all 8 banks
NG = M // GW  # 4 column groups

# kept for test.py compatibility (modes are baked into this kernel now)
MATMUL_MODE = "bf16"
REDUCE_MODE = "spass"
MM_W = 512

K = 20  # 5 augmented features x4 (bf16 hi/lo on both operands)
BF16 = mybir.dt.bfloat16
F32 = mybir.dt.float32
NEG_INF = -3.0e38

# column groups whose col-max fold chain runs on the Pool engine
POOL_GS = (2, 3)


def _build_nc(mode=MATMUL_MODE, reps=1, reduce_mode=REDUCE_MODE, mm_w=MM_W,
              pool_gs=POOL_GS):
    nc = bacc.Bacc("TRN2", target_bir_lowering=False, debug=False,
                   num_devices=NCORES)

    al = nc.dram_tensor("al", [B, K, NLOC], BF16, kind="ExternalInput")
    br = nc.dram_tensor("br", [B, K, M], BF16, kind="ExternalInput")
    # d1c[p, b*NRB+rb] = -min_m dist[b, rb*128+p, m]
    d1c = nc.dram_tensor("d1c", [128, B * NRB], BF16, kind="ExternalOutput")
    # d2c rows: negated partial col-mins, one per (batch, group, rb-pair).
    # row = b*16 + g*4 + pair. Host maxes over pairs and cores.
    d2c = nc.dram_tensor("d2c", [NG * 8, GW], BF16,
                         kind="ExternalOutput")

    MAX = mybir.AluOpType.max

    with tile.TileContext(nc) as tc, ExitStack() as ctx:
        sb = ctx.enter_context(tc.tile_pool(name="sb", bufs=1))
        ps = ctx.enter_context(tc.tile_pool(name="ps", bufs=2, space="PSUM"))
        scp = ctx.enter_context(tc.tile_pool(name="scp", bufs=2))
        hierp = ctx.enter_context(tc.tile_pool(name="hierp", bufs=2))
        outp = ctx.enter_context(tc.tile_pool(name="outp", bufs=2))

        def body():
            # ---- inputs -> SBUF ----
            al_sb, br_sb = {}, {}
            for b in range(B):
                t = sb.tile([K, NLOC], BF16, name=f"al{b}", tag=f"al{b}", bufs=2)
                nc.sync.dma_start(t[:], al.ap()[b])
                al_sb[b] = t
                t = sb.tile([K, M], BF16, name=f"br{b}", tag=f"br{b}", bufs=2)
                for g in range(NG):
                    nc.sync.dma_start(t[:, g * GW:(g + 1) * GW],
                                      br.ap()[b, :, g * GW:(g + 1) * GW])
                br_sb[b] = t

            # pre-warm the ACT function table during the input DMA so the
            # first real evacuation doesn't pay the ~2.7us table load
            warm = sb.tile([128, 16], BF16, name="warm", tag="warm")
            nc.vector.memset(warm[:], 0.0)
            nc.scalar.mul(warm[:], warm[:], -1.0)

            d1cols = sb.tile([128, B * NRB], BF16, name="d1cols", tag="d1cols")

            # ---- main loop: one [128, 8192] row-block at a time ----
            # Row blocks are processed in pairs: the even rb's evacuation
            # installs a fresh col-max accumulator per group (no DVE work),
            # the odd rb folds into it on DVE, then Pool immediately
            # partition-reduces the pair and the row is DMA'd out.
            cacc = {}
            for b in range(B):
                for rb in range(NRB):
                    lhsT = al_sb[b][:, rb * 128:(rb + 1) * 128]
                    pair = rb // 2
                    install = (rb % 2 == 0)
                    finish = (rb % 2 == 1)
                    scs = []
                    for g in range(NG):
                        pt = ps.tile([128, GW], F32, name="pt", tag="pt")
                        for j in range(GW // mm_w):
                            off = g * GW + j * mm_w
                            nc.tensor.matmul(
                                pt[:, j * mm_w:(j + 1) * mm_w],
                                lhsT,
                                br_sb[b][:, off:off + mm_w],
                            )
                        if install:
                            # negated evacuation doubles as accumulator.
                            # rb==0 runs on DVE (otherwise idle at start),
                            # the rest on ACT.
                            sc = scp.tile([128, GW], BF16, name=f"cacc{g}",
                                          tag=f"cacc{g}", bufs=3)
                            if rb == 0 and b == 0 and g >= 2:
                                nc.vector.tensor_scalar_mul(sc[:], pt[:], -1.0)
                            else:
                                nc.scalar.mul(sc[:], pt[:], -1.0)
                            cacc[g] = sc
                        else:
                            sc = scp.tile([128, GW], BF16, name=f"sc{g}",
                                          tag=f"sc{g}", bufs=2)
                            nc.scalar.mul(sc[:], pt[:], -1.0)
                            # DVE: fold odd rb into the pair accumulator
                            nc.vector.tensor_tensor(cacc[g][:], sc[:],
                                                    cacc[g][:], op=MAX)
                        scs.append(sc)

                    # DVE: row-max hierarchy for this row block
                    q1 = hierp.tile([128, GW], BF16, name="q1", tag="q1")
                    q2 = hierp.tile([128, GW], BF16, name="q2", tag="q2")
                    nc.vector.tensor_tensor(q1[:], scs[0][:], scs[1][:], op=MAX)
                    nc.vector.tensor_tensor(q2[:], scs[2][:], scs[3][:], op=MAX)
                    nc.vector.tensor_tensor(q1[:], q1[:], q2[:], op=MAX)
                    h1 = hierp.tile([128, 1024], BF16, name="h1", tag="h1")
                    nc.vector.tensor_tensor(h1[:], q1[:, 0:1024],
                                            q1[:, 1024:2048], op=MAX)
                    h2 = hierp.tile([128, 512], BF16, name="h2", tag="h2")
                    nc.vector.tensor_tensor(h2[:], h1[:, 0:512],
                                            h1[:, 512:1024], op=MAX)
                    h3 = hierp.tile([128, 256], BF16, name="h3", tag="h3")
                    nc.vector.tensor_tensor(h3[:], h2[:, 0:256],
                                            h2[:, 256:512], op=MAX)
                    col = b * NRB + rb
                    nc.vector.tensor_reduce(
                        d1cols[:, col:col + 1], h3[:],
                        axis=mybir.AxisListType.X, op=MAX)

                    if finish:
                        # Pool endgame for the completed chunk, interleaved
                        npair_b = 4
                        for g in range(NG):
                            pm = outp.tile([128, GW], BF16, name="pm",
                                           tag="pm")
                            nc.gpsimd.partition_all_reduce(
                                pm[:], cacc[g][:], channels=128,
                                reduce_op=bass_isa.ReduceOp.max)
                            row = b * 16 + g * npair_b + pair
                            nc.sync.dma_start(d2c.ap()[row], pm[0:1, :])

            nc.sync.dma_start(d1c.ap(), d1cols[:])

        if reps == 1:
            body()
        else:
            with tc.For_i(0, reps, 1):
                body()

    nc.compile()
    return nc


_NC_CACHE = {}


def _get_nc(mode=MATMUL_MODE, reps=1, reduce_mode=REDUCE_MODE, mm_w=MM_W):
    key = (mode, reps, reduce_mode, mm_w)
    if key not in _NC_CACHE:
        _NC_CACHE[key] = _build_nc(mode, reps, reduce_mode, mm_w)
    return _NC_CACHE[key]


def _lform(p):  # [B, n, 3] -> [B, 5, n]
    sq = (p * p).sum(-1)
    one = np.ones_like(sq)
    return np.stack([-2 * p[..., 0], -2 * p[..., 1], -2 * p[..., 2], sq, one],
                    axis=1)


def _rform(p):
    sq = (p * p).sum(-1)
    one = np.ones_like(sq)
    return np.stack([p[..., 0], p[..., 1], p[..., 2], one, sq], axis=1)


def _split_bf16(x):
    import ml_dtypes

    hi = x.astype(ml_dtypes.bfloat16).astype(np.float32)
    lo = (x - hi).astype(ml_dtypes.bfloat16).astype(np.float32)
    return hi, lo


def _pack(x, role):
    """f32 [B,5,n] -> matmul operand [B,20,n] bf16 (hi/lo product split)."""
    import ml_dtypes

    hi, lo = _split_bf16(x)
    if role == "l":
        out = np.concatenate([hi, hi, lo, lo], axis=1)
    else:
        out = np.concatenate([hi, lo, hi, lo], axis=1)
    return np.ascontiguousarray(out.astype(ml_dtypes.bfloat16))


def _make_in_maps(pc1, pc2, mode=MATMUL_MODE):
    L1 = _lform(pc1)
    R2 = _rform(pc2)
    L1p = _pack(L1, "l")
    brp = _pack(R2, "r")
    in_maps = []
    for c in range(NCORES):
        in_maps.append({
            "al": np.ascontiguousarray(L1p[:, :, c * NLOC:(c + 1) * NLOC]),
            "br": brp,
        })
    return in_maps


def kernel(pc1, pc2):
    pc1 = np.asarray(pc1, dtype=np.float32)
    pc2 = np.asarray(pc2, dtype=np.float32)
    assert pc1.shape == (B, N, 3) and pc2.shape == (B, M, 3)

    in_maps = _make_in_maps(pc1, pc2)
    nc = _get_nc()
    res = run_bass_kernel_spmd(nc, in_maps, list(range(NCORES)))

    # d1: negate, reassemble [B, N]
    d1 = np.empty((B, N), dtype=np.float64)
    for c in range(NCORES):
        d1c = np.asarray(res.results[c]["d1c"], dtype=np.float64)  # [128, B*NRB]
        for b in range(B):
            for rb in range(NRB):
                d1[b, c * NLOC + rb * 128:(c * NLOC) + (rb + 1) * 128] = \
                    -d1c[:, b * NRB + rb]

    # d2: max over cores and row-block chunks of negated partials, then negate
    d2n = np.full((B, M), -np.inf, dtype=np.float64)
    for c in range(NCORES):
        d2c = np.asarray(res.results[c]["d2c"], dtype=np.float64)
        for b, npair_b in ((0, 4), (1, 4)):
            part = d2c[b * 16:b * 16 + NG * npair_b]
            part = part.reshape(NG, npair_b, GW).max(axis=1)  # [NG, GW]
            np.maximum(d2n[b], part.reshape(M), out=d2n[b])
    d2 = -d2n

    out = d1.mean() + d2.mean()
    return np.float32(out)


# revision 19
# speedup vs baseline: 2.2604x; 1.2941x over previous
"""Chamfer distance kernel for 8 Trainium2 NeuronCores (Bass/Tile).

Problem: pc1, pc2: [2, 8192, 3] f32.
  dist[b,n,m] = ||pc1[b,n]-pc2[b,m]||^2
  out = mean_n(min_m dist) + mean_m(min_n dist)   (scalar f32)

Single-pass strategy (v2):
  * Augmented-matmul: dist[n,m] = L1[:,n] . R2[:,m] with K=20 bf16 hi/lo
    split (fp32-accurate distances straight into PSUM).
  * Each core owns 1/8 of pc1's rows and computes its [1024, 8192] block
    of the distance matrix ONCE (half the PE work of the two-pass scheme).
    - dist1 rows for the shard are complete: row-min over the free axis.
    - dist2 needs column mins: partial per core, combined on host.
  * All reduction work happens in NEGATED space (-d) because the gpsimd
    partition_all_reduce endgame only supports max:
    - ACT evacuates each PSUM tile with scale=-1 to bf16 SBUF (sc = -d);
      for even ("install") row blocks the evacuation doubles as a fresh
      per-pair col-max accumulator, so no separate DVE copy is needed.
    - DVE: folds the odd row block into the pair accumulator (bf16
      tensor_tensor max at the 2x rate) and computes the row-max hierarchy
      per row block (pairwise folds + one narrow reduce) -> -dist1 columns.
    - Pool: partition_all_reduce(max) per pair accumulator, interleaved
      with the main loop -> negated partial col-mins, one row per
      (batch, group, pair) DMA'd out.
  * Host: negate, max-combine pairs/cores for dist2, then means in fp64.

Engine budget per core (cost-model): ACT ~119us, DVE ~118us, Pool ~95us,
PE ~55us; sim total 137us (baseline two-pass was DVE-bound at ~297us).
"""

from contextlib import ExitStack

import numpy as np

import concourse.bass as bass
import concourse.tile as tile
from concourse import bacc, bass_isa, mybir
from concourse.bass_utils import run_bass_kernel_spmd

B = 2
N = 8192  # pc1 points per batch
M = 8192  # pc2 points per batch
NCORES = 8
NLOC = N // NCORES  # 1024 pc1 rows per core
NRB = NLOC // 128  # 8 row blocks per core
GW = 2048  # psum tile free width (4 banks); 2 bufs = all 8 banks
NG = M // GW  # 4 column groups

# kept for test.py compatibility (modes are baked into this kernel now)
MATMUL_MODE = "bf16"
REDUCE_MODE = "spass"
MM_W = 512

K = 20  # 5 augmented features x4 (bf16 hi/lo on both operands)
BF16 = mybir.dt.bfloat16
F32 = mybir.dt.float32
NEG_INF = -3.0e38

# column groups whose col-max fold chain runs on the Pool engine
POOL_GS = (2, 3)


def _build_nc(mode=MATMUL_MODE, reps=1, reduce_mode=REDUCE_MODE, mm_w=MM_W,
              pool_gs=POOL_GS):
    nc = bacc.Bacc("TRN2", target_bir_lowering=False, debug=False,
                   num_devices=NCORES)

    al = nc.dram_tensor("al", [B, K, NLOC], BF16, kind="ExternalInput")
    br = nc.dram_tensor("br", [B, K, M], BF16, kind="ExternalInput")
    # d1c[p, b*NRB+rb] = -min_m dist[b, rb*128+p, m]
    d1c = nc.dram_tensor("d1c", [128, B * NRB], BF16, kind="ExternalOutput")
    # d2c rows: negated partial col-mins, one per (batch, group, rb-pair).
    # row = b*16 + g*4 + pair. Host maxes over pairs and cores.
    d2c = nc.dram_tensor("d2c", [NG * 8, GW], BF16,
                         kind="ExternalOutput")

    MAX = mybir.AluOpType.max

    with tile.TileContext(nc) as tc, ExitStack() as ctx:
        sb = ctx.enter_context(tc.tile_pool(name="sb", bufs=1))
        ps = ctx.enter_context(tc.tile_pool(name="ps", bufs=2, space="PSUM"))
        scp = ctx.enter_context(tc.tile_pool(name="scp", bufs=2))
        hierp = ctx.enter_context(tc.tile_pool(name="hierp", bufs=3))
        outp = ctx.enter_context(tc.tile_pool(name="outp", bufs=3))

        def body():
            # ---- inputs -> SBUF ----
            al_sb, br_sb = {}, {}
            for b in range(B):
                t = sb.tile([K, NLOC], BF16, name=f"al{b}", tag=f"al{b}", bufs=2)
                nc.sync.dma_start(t[:], al.ap()[b])
                al_sb[b] = t
                t = sb.tile([K, M], BF16, name=f"br{b}", tag=f"br{b}", bufs=2)
                for g in range(NG):
                    nc.sync.dma_start(t[:, g * GW:(g + 1) * GW],
                                      br.ap()[b, :, g * GW:(g + 1) * GW])
                br_sb[b] = t

            # pre-warm the ACT function table during the input DMA so the
            # first real evacuation doesn't pay the ~2.7us table load
            warm = sb.tile([128, 16], BF16, name="warm", tag="warm")
            nc.vector.memset(warm[:], 0.0)
            nc.scalar.mul(warm[:], warm[:], -1.0)

            d1cols = sb.tile([128, B * NRB], BF16, name="d1cols", tag="d1cols")

            # ---- main loop: one [128, 8192] row-block at a time ----
            # Row blocks are processed in pairs: the even rb's evacuation
            # installs a fresh col-max accumulator per group (no DVE work),
            # the odd rb folds into it on DVE, then Pool immediately
            # partition-reduces the pair and the row is DMA'd out.
            cacc = {}
            for b in range(B):
                for rb in range(NRB):
                    lhsT = al_sb[b][:, rb * 128:(rb + 1) * 128]
                    pair = rb // 2
                    install = (rb % 2 == 0)
                    finish = (rb % 2 == 1)
                    scs = []
                    for g in range(NG):
                        pt = ps.tile([128, GW], F32, name="pt", tag="pt")
                        for j in range(GW // mm_w):
                            off = g * GW + j * mm_w
                            nc.tensor.matmul(
                                pt[:, j * mm_w:(j + 1) * mm_w],
                                lhsT,
                                br_sb[b][:, off:off + mm_w],
                            )
                        if install:
                            # negated evacuation doubles as accumulator.
                            # rb==0 runs on DVE (otherwise idle at start),
                            # the rest on ACT.
                            sc = scp.tile([128, GW], BF16, name=f"cacc{g}",
                                          tag=f"cacc{g}", bufs=3)
                            if rb == 0 and b == 0 and g >= 2:
                                nc.vector.tensor_scalar_mul(sc[:], pt[:], -1.0)
                            else:
                                nc.scalar.mul(sc[:], pt[:], -1.0)
                            cacc[g] = sc
                        else:
                            sc = scp.tile([128, GW], BF16, name=f"sc{g}",
                                          tag=f"sc{g}", bufs=2)
                            nc.scalar.mul(sc[:], pt[:], -1.0)
                            # DVE: fold odd rb into the pair accumulator
                            nc.vector.tensor_tensor(cacc[g][:], sc[:],
                                                    cacc[g][:], op=MAX)
                        scs.append(sc)

                    # DVE: row-max hierarchy for this row block
                    q1 = hierp.tile([128, GW], BF16, name="q1", tag="q1")
                    q2 = hierp.tile([128, GW], BF16, name="q2", tag="q2")
                    nc.vector.tensor_tensor(q1[:], scs[0][:], scs[1][:], op=MAX)
                    nc.vector.tensor_tensor(q2[:], scs[2][:], scs[3][:], op=MAX)
                    nc.vector.tensor_tensor(q1[:], q1[:], q2[:], op=MAX)
                    h1 = hierp.tile([128, 1024], BF16, name="h1", tag="h1")
                    nc.vector.tensor_tensor(h1[:], q1[:, 0:1024],
                                            q1[:, 1024:2048], op=MAX)
                    h2 = hierp.tile([128, 512], BF16, name="h2", tag="h2")
                    nc.vector.tensor_tensor(h2[:], h1[:, 0:512],
                                            h1[:, 512:1024], op=MAX)
                    h3 = hierp.tile([128, 256], BF16, name="h3", tag="h3")
                    nc.vector.tensor_tensor(h3[:], h2[:, 0:256],
                                            h2[:, 256:512], op=MAX)
                    col = b * NRB + rb
                    nc.vector.tensor_reduce(
                        d1cols[:, col:col + 1], h3[:],
                        axis=mybir.AxisListType.X, op=MAX)

                    if finish:
                        # Pool endgame for the completed chunk, interleaved
                        npair_b = 4
                        for g in range(NG):
                            pm = outp.tile([128, GW], BF16, name="pm",
                                           tag="pm")
                            nc.gpsimd.partition_all_reduce(
                                pm[:], cacc[g][:], channels=128,
                                reduce_op=bass_isa.ReduceOp.max)
                            row = b * 16 + g * npair_b + pair
                            nc.sync.dma_start(d2c.ap()[row], pm[0:1, :])

            nc.sync.dma_start(d1c.ap()[:, 0:NRB], d1cols[:, 0:NRB])
            nc.sync.dma_start(d1c.ap()[:, NRB:], d1cols[:, NRB:])

        if reps == 1:
            body()
        else:
            with tc.For_i(0, reps, 1):
                body()

    nc.compile()
    return nc


_NC_CACHE = {}


def _get_nc(mode=MATMUL_MODE, reps=1, reduce_mode=REDUCE_MODE, mm_w=MM_W):
    key = (mode, reps, reduce_mode, mm_w)
    if key not in _NC_CACHE:
        _NC_CACHE[key] = _build_nc(mode, reps, reduce_mode, mm_w)
    return _NC_CACHE[key]


def _lform(p):  # [B, n, 3] -> [B, 5, n]
    sq = (p * p).sum(-1)
    one = np.ones_like(sq)
    return np.stack([-2 * p[..., 0], -2 * p[..., 1], -2 * p[..., 2], sq, one],
                    axis=1)


def _rform(p):
    sq = (p * p).sum(-1)
    one = np.ones_like(sq)
    return np.stack([p[..., 0], p[..., 1], p[..., 2], one, sq], axis=1)


def _split_bf16(x):
    import ml_dtypes

    hi = x.astype(ml_dtypes.bfloat16).astype(np.float32)
    lo = (x - hi).astype(ml_dtypes.bfloat16).astype(np.float32)
    return hi, lo


def _pack(x, role):
    """f32 [B,5,n] -> matmul operand [B,20,n] bf16 (hi/lo product split)."""
    import ml_dtypes

    hi, lo = _split_bf16(x)
    if role == "l":
        out = np.concatenate([hi, hi, lo, lo], axis=1)
    else:
        out = np.concatenate([hi, lo, hi, lo], axis=1)
    return np.ascontiguousarray(out.astype(ml_dtypes.bfloat16))


def _make_in_maps(pc1, pc2, mode=MATMUL_MODE):
    L1 = _lform(pc1)
    R2 = _rform(pc2)
    L1p = _pack(L1, "l")
    brp = _pack(R2, "r")
    in_maps = []
    for c in range(NCORES):
        in_maps.append({
            "al": np.ascontiguousarray(L1p[:, :, c * NLOC:(c + 1) * NLOC]),
            "br": brp,
        })
    return in_maps


def kernel(pc1, pc2):
    pc1 = np.asarray(pc1, dtype=np.float32)
    pc2 = np.asarray(pc2, dtype=np.float32)
    assert pc1.shape == (B, N, 3) and pc2.shape == (B, M, 3)

    in_maps = _make_in_maps(pc1, pc2)
    nc = _get_nc()
    res = run_bass_kernel_spmd(nc, in_maps, list(range(NCORES)))

    # d1: negate, reassemble [B, N]
    d1 = np.empty((B, N), dtype=np.float64)
    for c in range(NCORES):
        d1c = np.asarray(res.results[c]["d1c"], dtype=np.float64)  # [128, B*NRB]
        for b in range(B):
            for rb in range(NRB):
                d1[b, c * NLOC + rb * 128:(c * NLOC) + (rb + 1) * 128] = \
                    -d1c[:, b * NRB + rb]

    # d2: max over cores and row-block chunks of negated partials, then negate
    d2n = np.full((B, M), -np.inf, dtype=np.float64)
    for c in range(NCORES):
        d2c = np.asarray(res.results[c]["d2c"], dtype=np.float64)
        for b, npair_b in ((0, 4), (1, 4)):
            part = d2c[b * 16:b * 16 + NG * npair_b]
            part = part.reshape(NG, npair_b, GW).max(axis=1)  # [NG, GW]
            np.maximum(d2n[b], part.reshape(M), out=d2n[b])
    d2 = -d2n

    out = d1.mean() + d2.mean()
    return np.float32(out)
